# revision 2
# baseline (speedup 1.0000x reference)
"""Trainium2 Bass kernel for nn_AttentionLayer_10995116278518.

Computes softmax(einsum('sbe,e->bs', embedded, attn[:300])
              + einsum('sbf,f->bs', lstm_outputs, attn[300:]), axis=1)
(the reference's mask is computed-but-discarded, so it is unused here).

Sharding: data-parallel over batch. Each of the 8 cores handles 8 of the
64 batch rows; no cross-device communication.

The kernel is HBM/DMA-engine-bandwidth bound (~36 MB/core at fp16, the
16 SDMA engines sustain ~26 GB/s each), so everything is built around
clean DMA streaming with minimal head/tail overhead:
  - host concatenates embedded+lstm features, casts to fp16 (validated
    against the 2e-2 tolerance; bf16 is NOT accurate enough), and lays
    the shard out feature-major, pre-permuted so every DMA is a single
    fully-contiguous read with 32 KB-per-partition descriptor runs.
  - the stream is 15 large DMAs (mostly 4 MB groups of four 128-feature
    chunks) alternating between the two HWDGE rings; fewer DMAs means
    fewer completion-semaphore descriptors (which pile onto one SDMA
    engine and straggle the stream end) and better per-engine
    descriptor efficiency.
  - TensorE does the dots with a windowed-attn lhsT: tile T[k, c, j]
    holds attn chunk c at column 8, zeros elsewhere; the lhsT for
    (chunk c, batch row b) is the 8-column window T[:, c, 8-b : 16-b],
    which places the attn column at output row b. ALL matmuls therefore
    accumulate into a single PSUM bank whose partition b holds batch
    row b's logits -- no PSUM->SBUF copies, no scatter DMAs.
  - the final feature chunk is DMA'd in four column-quarters (two batch
    rows each) so the post-stream PE drain is ~2 matmuls, and the
    softmax (max/exp/sum/scale) reads the PSUM bank directly.
"""

import sys

import numpy as np

try:
    import concourse.bass as bass
except ImportError:  # stand-alone grading dir: the runtime lives here
    sys.path.insert(0, "/opt/trn_rl_repo")
    import concourse.bass as bass

import concourse.bacc as bacc
import concourse.tile as tile
from concourse import mybir
from concourse.bass_utils import run_bass_kernel_spmd

SEQ = 512
BATCH = 64
EMB = 300
LSTM = 4096
D = EMB + LSTM  # 4396
N_CORES = 8
BLOC = BATCH // N_CORES  # 8 batch rows per core
P = 128
RC = BLOC * SEQ  # 4096 columns (b-major) per chunk
NCH = (D + P - 1) // P  # 35 feature chunks: 34 full + 1 of 44
KLAST = D - (NCH - 1) * P  # 44

F32 = mybir.dt.float32
F16 = mybir.dt.float16

# windowed-attn tile: T[k, c, j], attn chunk c at column WCOL, so the
# [*, 8] window starting at WCOL-b has the attn column at index b
WSTRIDE = 16
WCOL = 8

# stream schedule: full chunks 0..33 grouped, then the 44-row partial
# chunk (34), then chunk 33 last, split in column-quarters.
GROUPS = [list(range(4 * g, 4 * g + 4)) for g in range(7)] + [[28, 29, 30]]
SINGLES = [31, 32]
CPART = NCH - 1  # 34
CLAST = 33


def _build() -> bass.Bass:
    nc = bacc.Bacc()
    # flat fp16 stream, pre-permuted on host to match the DMA schedule
    x = nc.declare_dram_parameter("x", [D * RC], F16, isOutput=False)
    attn_win = nc.declare_dram_parameter(
        "attn_win", [P, (NCH + 1) * WSTRIDE], F16, isOutput=False
    )
    out = nc.declare_dram_parameter("out", [BLOC, SEQ], F16, isOutput=True)

    # flat offsets of each transfer in x (elements)
    offs = {}
    pos = 0
    for gi, g in enumerate(GROUPS):
        offs[("g", gi)] = pos
        pos += len(g) * P * RC
    for c in SINGLES:
        offs[("s", c)] = pos
        pos += P * RC
    offs[("s", CPART)] = pos
    pos += KLAST * RC
    for q in range(4):
        offs[("q", q)] = pos
        pos += P * (RC // 4)
    assert pos == D * RC

    with tile.TileContext(nc) as tc:
        with (
            tc.tile_pool(name="singles", bufs=1) as singles,
            tc.tile_pool(name="gpool", bufs=4) as gpool,
            tc.tile_pool(name="tpool", bufs=4) as tpool,
            tc.tile_pool(name="psum", bufs=1, space="PSUM") as psum_pool,
        ):
            sb_attn = singles.tile([P, NCH + 1, WSTRIDE], F16)
            nc.scalar.dma_start(out=sb_attn, in_=attn_win[:, :])

            ps = psum_pool.tile([BLOC, SEQ], F32)

            # issue the whole stream up front, alternating HWDGE rings;
            # the tile pools throttle via buffer-recycle semaphores
            gtiles = []
            for gi, g in enumerate(GROUPS):
                n = len(g) * P * RC
                gt = gpool.tile([P, len(g) * RC], F16, tag="g")
                eng = nc.sync if gi % 2 == 0 else nc.scalar
                a = offs[("g", gi)]
                eng.dma_start(out=gt, in_=x[a : a + n])
                gtiles.append(gt)
            stiles = {}
            for i, c in enumerate(SINGLES):
                st = tpool.tile([P, RC], F16, tag="t")
                eng = nc.sync if i % 2 == 0 else nc.scalar
                a = offs[("s", c)]
                eng.dma_start(out=st, in_=x[a : a + P * RC])
                stiles[c] = st
            # 44-row partial chunk
            pt = tpool.tile([P, RC], F16, tag="t")
            a = offs[("s", CPART)]
            nc.sync.dma_start(out=pt[0:KLAST, :], in_=x[a : a + KLAST * RC])
            stiles[CPART] = pt
            # final chunk in column-quarters (two batch rows each)
            lt = tpool.tile([P, RC], F16, tag="t")
            for q in range(4):
                a = offs[("q", q)]
                eng = nc.scalar if q % 2 == 0 else nc.sync
                eng.dma_start(
                    out=lt[:, q * (RC // 4) : (q + 1) * (RC // 4)],
                    in_=x[a : a + P * (RC // 4)],
                )
            stiles[CLAST] = lt

            first = True

            def chunk_mms(c, xt, col0, kp, dummy):
                nonlocal first
                for b in range(BLOC):
                    nc.tensor.matmul(
                        out=ps,
                        lhsT=sb_attn[0:kp, c, WCOL - b : WCOL - b + BLOC],
                        rhs=xt[0:kp, col0 + b * SEQ : col0 + (b + 1) * SEQ],
                        start=first,
                        stop=False,
                        skip_group_check=True,
                    )
                    first = False
                if dummy:
                    # zero-weight matmul: adds 0 but keeps the PE HAM
                    # clock gate warm through DMA-wait gaps
                    nc.tensor.matmul(
                        out=ps,
                        lhsT=sb_attn[0:kp, NCH, 0:BLOC],
                        rhs=xt[0:kp, col0 : col0 + SEQ],
                        start=False,
                        stop=False,
                        skip_group_check=True,
                    )

            for gi, g in enumerate(GROUPS):
                for j, c in enumerate(g):
                    chunk_mms(c, gtiles[gi], j * RC, P, dummy=True)
            for c in SINGLES:
                chunk_mms(c, stiles[c], 0, P, dummy=True)
            chunk_mms(CPART, stiles[CPART], 0, KLAST, dummy=False)
            # final chunk: matmuls per column-quarter chase the 4 DMAs
            for b in range(BLOC):
                nc.tensor.matmul(
                    out=ps,
                    lhsT=sb_attn[0:P, CLAST, WCOL - b : WCOL - b + BLOC],
                    rhs=stiles[CLAST][0:P, b * SEQ : (b + 1) * SEQ],
                    start=False,
                    stop=(b == BLOC - 1),
                    skip_group_check=True,
                )

            # softmax along s (free axis), reading logits straight from
            # PSUM: partition b of the bank holds batch row b
            nm = singles.tile([BLOC, 1], F32)
            ssum = singles.tile([BLOC, 1], F32)
            rec = singles.tile([BLOC, 1], F32)
            expt = singles.tile([BLOC, SEQ], F32)
            res = singles.tile([BLOC, SEQ], F16)
            nc.vector.tensor_reduce(
                out=nm,
                in_=ps,
                axis=mybir.AxisListType.X,
                op=mybir.AluOpType.max,
                negate=True,
            )
            nc.scalar.activation(
                out=expt,
                in_=ps,
                func=mybir.ActivationFunctionType.Exp,
                bias=nm,
                scale=1.0,
                accum_out=ssum,
            )
            nc.vector.reciprocal(rec, ssum)
            nc.vector.tensor_scalar_mul(res, expt, rec)
            nc.sync.dma_start(out=out[:, :], in_=res)

    nc.compile()
    return nc


_NC_CACHE = None


def _get_nc() -> bass.Bass:
    global _NC_CACHE
    if _NC_CACHE is None:
        _NC_CACHE = _build()
    return _NC_CACHE


def _make_in_maps(embedded, lstm_outputs, attn):
    embedded = np.asarray(embedded, dtype=np.float32)
    lstm_outputs = np.asarray(lstm_outputs, dtype=np.float32)
    attn = np.asarray(attn, dtype=np.float32).astype(np.float16)
    # [S, B, F] -> [s, core, b, F]
    emb4 = embedded.reshape(SEQ, N_CORES, BLOC, EMB)
    lst4 = lstm_outputs.reshape(SEQ, N_CORES, BLOC, LSTM)
    att_win = np.zeros((P, NCH + 1, WSTRIDE), dtype=np.float16)
    for c in range(NCH):
        kp = P if c < NCH - 1 else KLAST
        att_win[:kp, c, WCOL] = attn[c * P : c * P + kp]
    att_flat = att_win.reshape(P, (NCH + 1) * WSTRIDE)
    in_maps = []
    for i in range(N_CORES):
        xs = np.empty((D, RC), dtype=np.float16)
        # [s, b, F] -> [F, b, s] -> [F, b*512+s]
        xs[:EMB] = emb4[:, i].transpose(2, 1, 0).reshape(EMB, RC)
        xs[EMB:] = lst4[:, i].transpose(2, 1, 0).reshape(LSTM, RC)
        pieces = []
        for g in GROUPS:
            c0, n = g[0], len(g)
            pieces.append(
                xs[c0 * P : (c0 + n) * P]
                .reshape(n, P, RC)
                .transpose(1, 0, 2)
                .ravel()
            )
        for c in SINGLES:
            pieces.append(xs[c * P : (c + 1) * P].ravel())
        pieces.append(xs[CPART * P : CPART * P + KLAST].ravel())
        x33 = xs[CLAST * P : (CLAST + 1) * P].reshape(P, 4, RC // 4)
        for q in range(4):
            pieces.append(x33[:, q, :].ravel())
        in_maps.append(
            {"x": np.concatenate(pieces), "attn_win": att_flat}
        )
    return in_maps


def _run(embedded, lstm_outputs, attn, trace=False, **spmd_kwargs):
    nc = _get_nc()
    in_maps = _make_in_maps(embedded, lstm_outputs, attn)
    r = run_bass_kernel_spmd(
        nc, in_maps, core_ids=list(range(N_CORES)), trace=trace, **spmd_kwargs
    )
    out = np.concatenate([r.results[i]["out"] for i in range(N_CORES)], axis=0)
    return out, r


def kernel(embedded, lstm_outputs, attn, mask=None, **_ignored) -> np.ndarray:
    out, _ = _run(embedded, lstm_outputs, attn, trace=False)
    return out.astype(np.float32)
